# revision 8
# baseline (speedup 1.0000x reference)
"""ODE-RNN (nn_ODERNN) Trainium2 Bass kernel, V2.

Data-parallel over batch across 8 NeuronCores; per core 136 rows split in
G=2 chunks of 68 whose step-chains run phase-shifted (chunk B's ODE under
chunk A's GRU). All matmuls fp32 (recurrence precision), state transposed
[DRNN=128 part, rows free].

Latency-oriented structure vs V1:
- Manual PSUM bank layout (3 banks per chunk). z1-lo/hi pair in two
  adjacent banks -> single strided-AP ACT per ODE layer.
- Sa2 = sum_k a2_k accumulated in SBUF; GRU gate h-sides delivered as
  X^T h1 = X^T hprev + (DT*w2@X)^T Sa2, so the r/z gates of GRU0 close
  right after Sa2 (no wait on h1 materialization).
- GRU1 x-side via h2b; next-step z1 init from hn (telescoped W20 form
  retained inside the ODE).
- On-path elementwise on GPSIMD(Pool), off-path on DVE; strict per-bank
  sequential accumulation-group discipline (interleaved open groups in one
  bank corrupt PSUM - verified on this stack).
- Emission is software-pipelined per half-step: ODE(chunk B, s) interleaved
  with GRU(chunk A, s') so each engine's in-order stream matches the
  intended schedule.
"""

import os
import numpy as np
import ml_dtypes

B, S, P, J = 32, 128, 2, 17
DIN, DOUT, DRNN, DHID = 2, 3, 128, 256
N = P * J            # 34
DT = 0.1
K = 4                # Euler steps
NCORES = 8
BL = B // NCORES     # 4 batches per core
R = BL * N           # 136 rows per core
RG = R // 2          # 68 rows per chunk

NSTEPS = int(os.environ.get("ODERNN_STEPS", S))
SR = NSTEPS * R

BF16 = ml_dtypes.bfloat16

_prog_cache = {}


def _build_program(flags):
    import concourse.bass as bass
    import concourse.tile as tile
    import concourse.mybir as mybir
    from concourse import bacc

    (fb01, fb2, frz0, fg0h, frz1, fg1h, fbout) = flags

    dt = mybir.dt
    f32 = dt.float32
    bf16 = dt.bfloat16
    Alu = mybir.AluOpType
    Act = mybir.ActivationFunctionType

    nc = bacc.Bacc("TRN2", target_bir_lowering=False)

    # ---- DRAM I/O ----
    d_xm = nc.dram_tensor("xm", [DIN, SR], f32, kind="ExternalInput")
    d_mbc = nc.dram_tensor("mbc", [DRNN, SR], bf16, kind="ExternalInput")
    d_h0f = nc.dram_tensor("h0f", [DRNN, R], f32, kind="ExternalInput")
    d_w0 = nc.dram_tensor("w0", [DRNN, DHID], f32, kind="ExternalInput")
    d_w1a = nc.dram_tensor("w1a", [128, DHID], f32, kind="ExternalInput")
    d_w1b = nc.dram_tensor("w1b", [128, DHID], f32, kind="ExternalInput")
    d_w2a = nc.dram_tensor("w2a", [128, DRNN], f32, kind="ExternalInput")
    d_w2b = nc.dram_tensor("w2b", [128, DRNN], f32, kind="ExternalInput")
    d_W20a = nc.dram_tensor("W20a", [128, DHID], f32, kind="ExternalInput")
    d_W20b = nc.dram_tensor("W20b", [128, DHID], f32, kind="ExternalInput")
    d_C0a = nc.dram_tensor("C0a", [128, 2 * DRNN], f32, kind="ExternalInput")
    d_C0b = nc.dram_tensor("C0b", [128, 2 * DRNN], f32, kind="ExternalInput")
    d_wih0 = nc.dram_tensor("wih0", [DIN, 3 * DRNN], f32, kind="ExternalInput")
    d_whh0 = nc.dram_tensor("whh0", [DRNN, 3 * DRNN], f32, kind="ExternalInput")
    d_wih1 = nc.dram_tensor("wih1", [DRNN, 3 * DRNN], f32, kind="ExternalInput")
    d_whh1 = nc.dram_tensor("whh1", [DRNN, 3 * DRNN], f32, kind="ExternalInput")
    d_wout = nc.dram_tensor("wout", [DRNN, DOUT], f32, kind="ExternalInput")
    d_bias = nc.dram_tensor("biaspk", [DRNN, 24], f32, kind="ExternalInput")
    d_y = nc.dram_tensor("y", [NSTEPS, DOUT, R], f32, kind="ExternalOutput")

    with tile.TileContext(nc) as tc:
        wp = tc.alloc_tile_pool(name="wconst", bufs=1)
        st = tc.alloc_tile_pool(name="state", bufs=1)
        wk = tc.alloc_tile_pool(name="work", bufs=2)

        def load(pool, dram, shape, dtype, name):
            t = pool.tile(shape, dtype, tag=name, name=name)
            nc.sync.dma_start(out=t[:], in_=dram[:])
            return t

        xm = load(wp, d_xm, [DIN, SR], f32, "xm")
        mbc = load(wp, d_mbc, [DRNN, SR], bf16, "mbc")
        h0f = load(wp, d_h0f, [DRNN, R], f32, "h0f")
        w0 = load(wp, d_w0, [DRNN, DHID], f32, "w0")
        w1a = load(wp, d_w1a, [128, DHID], f32, "w1a")
        w1b = load(wp, d_w1b, [128, DHID], f32, "w1b")
        w2a = load(wp, d_w2a, [128, DRNN], f32, "w2a")
        w2b = load(wp, d_w2b, [128, DRNN], f32, "w2b")
        W20a = load(wp, d_W20a, [128, DHID], f32, "W20a")
        W20b = load(wp, d_W20b, [128, DHID], f32, "W20b")
        C0a = load(wp, d_C0a, [128, 2 * DRNN], f32, "C0a")
        C0b = load(wp, d_C0b, [128, 2 * DRNN], f32, "C0b")
        wih0 = load(wp, d_wih0, [DIN, 3 * DRNN], f32, "wih0")
        whh0 = load(wp, d_whh0, [DRNN, 3 * DRNN], f32, "whh0")
        wih1 = load(wp, d_wih1, [DRNN, 3 * DRNN], f32, "wih1")
        whh1 = load(wp, d_whh1, [DRNN, 3 * DRNN], f32, "whh1")
        wout = load(wp, d_wout, [DRNN, DOUT], f32, "wout")
        bias = load(wp, d_bias, [DRNN, 24], f32, "biaspk")

        MM = nc.tensor.matmul
        ACT = nc.scalar.activation
        V = nc.vector
        G = nc.gpsimd

        # ---- manual PSUM layout: 3 banks per chunk ----
        # pZ[c]: [128, 2, 512] spanning banks (3c, 3c+1)
        #   [:, :, 0:RG]      z1 pair (lo bank A, hi bank B) - telescope
        #   [:, :, RG:2RG]    prz pair (prz0 then prz1, sequential reuse)
        #   [:, 1, 2RG:3RG]   gi1g (bank B)
        # pM[c]: [128, 512] at bank 3c+2
        #   [0:2RG]    z2 pair (lo/hi groups kept sequential)
        #   [2RG:3RG]  gi0g | [3RG:4RG] hd | [4RG:5RG] hg0
        #   [5RG:6RG]  hg1  | [6RG:7RG] py (partitions 0:DOUT)
        pZ = [nc.place_psum_tensor(f"pZ{c}", [128, 2, 512], f32, bank=3 * c)
              for c in range(2)]
        pM = [nc.place_psum_tensor(f"pM{c}", [128, 512], f32, bank=3 * c + 2)
              for c in range(2)]

        # ---- persistent SBUF state per chunk ----
        def stile(name, shape=None):
            return st.tile(shape or [128, 2, RG], f32, tag=name, name=name)

        hn_t = [stile(f"hn{c}", [128, RG]) for c in range(2)]
        h1f_t = [stile(f"h1f{c}", [128, RG]) for c in range(2)]
        sa2_t = [stile(f"sa2{c}") for c in range(2)]

        def wtile(name, shape=None):
            return wk.tile(shape or [128, RG], f32, tag=name, name=name)

        hprev = [h0f[:, c * RG:(c + 1) * RG] for c in range(2)]

        # per-chunk live tiles for the GRU phase (dict refreshed per step)
        live = [dict(), dict()]

        def emit_z1_init(c, hsrc):
            """Open z1 pair groups for (c, next step) from SBUF state hsrc."""
            z = pZ[c]
            MM(z[:, 0, 0:RG], w0[:, 0:128], hsrc, start=True, stop=False)
            MM(z[:, 1, 0:RG], w0[:, 128:256], hsrc, start=True, stop=False)

        def ode_k_front(c, s, k):
            """ODE layer front half: a1 ACT + z2 matmuls."""
            z = pZ[c]
            m = pM[c]
            a1 = wtile(f"a1_{c}", [128, 2, RG])
            if not fb01:
                ACT(a1[:], z[:, :, 0:RG], Act.Tanh)
            else:
                ACT(a1[:, 0, :], z[:, 0, 0:RG], Act.Tanh,
                    bias=bias[:, 16 + 2 * k:17 + 2 * k])
                ACT(a1[:, 1, :], z[:, 1, 0:RG], Act.Tanh,
                    bias=bias[:, 17 + 2 * k:18 + 2 * k])
            # z2 pair: sequential groups in one bank
            MM(m[0:128, 0:RG], w1a[:, 0:128], a1[:, 0, :], start=True, stop=False)
            MM(m[0:128, 0:RG], w1b[:, 0:128], a1[:, 1, :], start=False, stop=True)
            MM(m[0:128, RG:2 * RG], w1a[:, 128:256], a1[:, 0, :], start=True, stop=False)
            MM(m[0:128, RG:2 * RG], w1b[:, 128:256], a1[:, 1, :], start=False, stop=True)

        def ode_k_back(c, s, k):
            """ODE layer back half: a2 ACT + telescope matmuls + Sa2."""
            z = pZ[c]
            m = pM[c]
            last = k == K - 1
            a2 = wtile(f"a2_{c}", [128, 2, RG])
            if not fb01:
                ACT(a2[:], m[0:128, 0:2 * RG], Act.Tanh)
            else:
                ACT(a2[:, 0, :], m[0:128, 0:RG], Act.Tanh, bias=bias[:, 2:3])
                ACT(a2[:, 1, :], m[0:128, RG:2 * RG], Act.Tanh, bias=bias[:, 3:4])
            # telescope z1 += W20^T a2 (k < K-1); stop at k == K-2
            if not last:
                stop = k == K - 2
                MM(z[:, 0, 0:RG], W20a[:, 0:128], a2[:, 0, :], start=False, stop=False)
                MM(z[:, 0, 0:RG], W20b[:, 0:128], a2[:, 1, :], start=False, stop=stop)
                MM(z[:, 1, 0:RG], W20a[:, 128:256], a2[:, 0, :], start=False, stop=False)
                MM(z[:, 1, 0:RG], W20b[:, 128:256], a2[:, 1, :], start=False, stop=stop)
            # Sa2 accumulation: k0 copy (Pool), k1/k2 add (Pool), k3 add (DVE)
            sa2 = sa2_t[c]
            if k == 0:
                G.tensor_copy(sa2[:], a2[:])
            elif k < K - 1:
                G.tensor_add(sa2[:], sa2[:], a2[:])
            else:
                V.tensor_add(sa2[:], sa2[:], a2[:])

        def ode_prefire(c, s):
            """prz0 x/hprev parts (group opens; closed later by comps) and
            gi0g. Emitted at ODE k3 of (c, s); consumed by GRU(c, s)."""
            z = pZ[c]
            m = pM[c]
            c0 = s * R + c * RG
            xsl = xm[:, c0:c0 + RG]
            hp = hprev[c]
            # gi0g = wih0_g^T x (closed group, bank 3c+2 after z2 k3)
            MM(m[0:128, 2 * RG:3 * RG], wih0[:, 256:384], xsl, start=True, stop=True)
            # prz pair groups open: x part + hprev part
            MM(z[:, 0, RG:2 * RG], wih0[:, 0:128], xsl, start=True, stop=False)
            MM(z[:, 0, RG:2 * RG], whh0[:, 0:128], hp, start=False, stop=False)
            MM(z[:, 1, RG:2 * RG], wih0[:, 128:256], xsl, start=True, stop=False)
            MM(z[:, 1, RG:2 * RG], whh0[:, 128:256], hp, start=False, stop=False)

        # --- GRU phase op groups, assigned to slots 0..3 of the other
        # chunk's ODE k loop. Each slot split into (PE matmuls, rest) so the
        # in-order PE stream runs the GRU matmuls (deps already satisfied)
        # while the ODE chunk waits on its ACT. ---
        def gru_slot0_mm(c, s):
            z = pZ[c]
            m = pM[c]
            sa2 = sa2_t[c]
            # close prz0 pair with Sa2 composites (on path)
            MM(z[:, 0, RG:2 * RG], C0a[:, 0:128], sa2[:, 0, :], start=False, stop=False)
            MM(z[:, 0, RG:2 * RG], C0b[:, 0:128], sa2[:, 1, :], start=False, stop=True)
            MM(z[:, 1, RG:2 * RG], C0a[:, 128:256], sa2[:, 0, :], start=False, stop=False)
            MM(z[:, 1, RG:2 * RG], C0b[:, 128:256], sa2[:, 1, :], start=False, stop=True)
            # hd = DT*w2^T Sa2 (closed group)
            MM(m[0:128, 3 * RG:4 * RG], w2a[:], sa2[:, 0, :], start=True, stop=False)
            MM(m[0:128, 3 * RG:4 * RG], w2b[:], sa2[:, 1, :], start=False, stop=True)

        def gru_slot0_rest(c, s):
            z = pZ[c]
            m = pM[c]
            rz0 = wtile(f"rz0_{c}", [128, 2, RG])
            if not frz0:
                ACT(rz0[:], z[:, :, RG:2 * RG], Act.Sigmoid)
            else:
                ACT(rz0[:, 0, :], z[:, 0, RG:2 * RG], Act.Sigmoid, bias=bias[:, 5:6])
                ACT(rz0[:, 1, :], z[:, 1, RG:2 * RG], Act.Sigmoid, bias=bias[:, 6:7])
            # h1 = hprev + hd (+DT*b2)  (DVE, off path)
            h1f = h1f_t[c]
            V.scalar_tensor_tensor(h1f[:], m[0:128, 3 * RG:4 * RG], bias[:, 4:5],
                                   hprev[c], op0=Alu.add, op1=Alu.add)
            live[c]["rz0"] = rz0

        def gru_slot1_mm(c, s):
            z = pZ[c]
            m = pM[c]
            h1f = h1f_t[c]
            # h1-direct gate MMs (closed groups / prz1 pair opens)
            MM(m[0:128, 4 * RG:5 * RG], whh0[:, 256:384], h1f[:], start=True, stop=True)  # hg0
            MM(m[0:128, 5 * RG:6 * RG], whh1[:, 256:384], h1f[:], start=True, stop=True)  # hg1
            MM(z[:, 0, RG:2 * RG], whh1[:, 0:128], h1f[:], start=True, stop=False)   # prz1 lo
            MM(z[:, 1, RG:2 * RG], whh1[:, 128:256], h1f[:], start=True, stop=False)  # prz1 hi
            # y projection: py = wout^T h1 (+bout)
            MM(m[0:DOUT, 6 * RG:7 * RG], wout[:], h1f[:], start=True, stop=True)

        def gru_slot1_rest(c, s):
            m = pM[c]
            h1f = h1f_t[c]
            rz0 = live[c]["rz0"]
            # s1 = r0 * hg0 ; np0 = s1 + gi0g ; n0 = tanh (Pool, Pool, ACT)
            s1 = wtile(f"s1_{c}")
            V.tensor_mul(s1[:], rz0[:, 0, :], m[0:128, 4 * RG:5 * RG])
            np0 = wtile(f"np0_{c}")
            V.tensor_add(np0[:], s1[:], m[0:128, 2 * RG:3 * RG])
            n0 = wtile(f"n0_{c}")
            if not fg0h:
                ACT(n0[:], np0[:], Act.Tanh)
            else:
                ACT(n0[:], np0[:], Act.Tanh, bias=bias[:, 7:8])
            # off-path: u0 = 1 - z0 ; t0 = z0 * h1 (DVE)
            u0 = wtile(f"u0_{c}")
            G.tensor_scalar(u0[:], rz0[:, 1, :], -1.0, 1.0, op0=Alu.mult, op1=Alu.add)
            t0 = wtile(f"t0_{c}")
            G.tensor_mul(t0[:], rz0[:, 1, :], h1f[:])
            # v0 = u0 * n0 ; h2b = v0 + t0 (Pool, on path)
            v0 = wtile(f"v0_{c}")
            G.tensor_mul(v0[:], u0[:], n0[:])
            h2b = wtile(f"h2b_{c}")
            G.tensor_add(h2b[:], v0[:], t0[:])
            live[c]["h2b"] = h2b

        def gru_slot2_mm(c, s):
            z = pZ[c]
            h2b = live[c]["h2b"]
            # close prz1 pair with wih1^T h2b ; gi1g closed group
            MM(z[:, 0, RG:2 * RG], wih1[:, 0:128], h2b[:], start=False, stop=True)
            MM(z[:, 1, RG:2 * RG], wih1[:, 128:256], h2b[:], start=False, stop=True)
            MM(z[:, 1, 2 * RG:3 * RG], wih1[:, 256:384], h2b[:], start=True, stop=True)

        def gru_slot2_rest(c, s):
            z = pZ[c]
            m = pM[c]
            rz1 = wtile(f"rz1_{c}", [128, 2, RG])
            if not frz1:
                ACT(rz1[:], z[:, :, RG:2 * RG], Act.Sigmoid)
            else:
                ACT(rz1[:, 0, :], z[:, 0, RG:2 * RG], Act.Sigmoid, bias=bias[:, 9:10])
                ACT(rz1[:, 1, :], z[:, 1, RG:2 * RG], Act.Sigmoid, bias=bias[:, 10:11])
            # s2 = r1 * hg1 ; np1 = s2 + gi1g (Pool)
            s2 = wtile(f"s2_{c}")
            V.tensor_mul(s2[:], rz1[:, 0, :], m[0:128, 5 * RG:6 * RG])
            np1 = wtile(f"np1_{c}")
            V.tensor_add(np1[:], s2[:], z[:, 1, 2 * RG:3 * RG])
            # y out copy + DMA (off path, DVE)
            ysl = wtile(f"ysl_{c}", [DOUT, RG])
            if not fbout:
                V.tensor_copy(ysl[:], m[0:DOUT, 6 * RG:7 * RG])
            else:
                ACT(ysl[:], m[0:DOUT, 6 * RG:7 * RG], Act.Identity,
                    bias=bias[0:DOUT, 15:16])
            nc.sync.dma_start(out=d_y[s, :, c * RG:(c + 1) * RG], in_=ysl[:])
            live[c]["rz1"] = rz1
            live[c]["np1"] = np1

        def gru_slot3_mm(c, s):
            pass

        def gru_slot3_rest(c, s):
            rz1 = live[c]["rz1"]
            np1 = live[c]["np1"]
            h1f = h1f_t[c]
            c0 = s * R + c * RG
            msl = mbc[:, c0:c0 + RG]
            n1 = wtile(f"n1_{c}")
            if not fg1h:
                ACT(n1[:], np1[:], Act.Tanh)
            else:
                ACT(n1[:], np1[:], Act.Tanh, bias=bias[:, 11:12])
            # off-path DVE: u1, g, tg, hm
            u1 = wtile(f"u1_{c}")
            G.tensor_scalar(u1[:], rz1[:, 1, :], -1.0, 1.0, op0=Alu.mult, op1=Alu.add)
            g = wtile(f"g_{c}")
            G.tensor_mul(g[:], u1[:], msl)
            tg = wtile(f"tg_{c}")
            G.tensor_mul(tg[:], g[:], h1f[:])
            hm = wtile(f"hm_{c}")
            G.tensor_sub(hm[:], h1f[:], tg[:])
            # on-path Pool: vg = g*n1 ; hn = vg + hm
            vg = wtile(f"vg_{c}")
            G.tensor_mul(vg[:], g[:], n1[:])
            hn = hn_t[c]
            G.tensor_add(hn[:], vg[:], hm[:])
            hprev[c] = hn[:]

        GRU_MM = (gru_slot0_mm, gru_slot1_mm, gru_slot2_mm, gru_slot3_mm)
        GRU_REST = (gru_slot0_rest, gru_slot1_rest, gru_slot2_rest,
                    gru_slot3_rest)

        def emit_half(ode_cs, gru_cs):
            for k in range(K):
                if gru_cs is not None:
                    GRU_MM[k](*gru_cs)
                if ode_cs is not None:
                    c, s = ode_cs
                    if k == K - 1:
                        ode_prefire(c, s)
                    ode_k_front(c, s, k)
                if gru_cs is not None:
                    GRU_REST[k](*gru_cs)
                if ode_cs is not None:
                    ode_k_back(*ode_cs, k)
            if gru_cs is not None and gru_cs[1] < NSTEPS - 1:
                # open next-step z1 pair from hn; emitted last so the PE
                # stream isn't head-of-line blocked waiting for hn
                emit_z1_init(gru_cs[0], hprev[gru_cs[0]])

        # ---- schedule ----
        emit_z1_init(0, hprev[0])
        emit_z1_init(1, hprev[1])
        emit_half((0, 0), None)
        prev = (0, 0)
        for hs in range(1, 2 * NSTEPS):
            s, c = divmod(hs, 2)
            emit_half((c, s), prev)
            prev = (c, s)
        emit_half(None, prev)

        wk.release()
        st.release()
        wp.release()

    nc.compile()
    return nc


def _to_bf(x):
    return np.ascontiguousarray(x.astype(BF16))


def _prep(inputs):
    """Host-side prep: shard over batch, transpose layouts, pack weights."""
    x2d = np.asarray(inputs["x2d"], np.float32)
    mask = np.asarray(inputs["mask"])
    g = lambda n: np.asarray(inputs[n], np.float32)
    w0, b0 = g("ode_w0"), g("ode_b0")
    w1, b1 = g("ode_w1"), g("ode_b1")
    w2, b2 = g("ode_w2"), g("ode_b2")
    wih0, whh0 = g("wih0"), g("whh0")
    bih0, bhh0 = g("bih0"), g("bhh0")
    wih1, whh1 = g("wih1"), g("whh1")
    bih1, bhh1 = g("bih1"), g("bhh1")
    wout, bout = g("wout"), g("bout")
    h0 = g("h0")

    mf = mask.astype(np.float32)
    xs = (x2d * mf).reshape(B, S, N, DIN)[:, :NSTEPS]
    ms = mf.reshape(B, S, N)[:, :NSTEPS]

    W20 = (DT * (w2.astype(np.float64) @ w0.astype(np.float64))).astype(np.float32)
    # composite for GRU0 r/z gates through Sa2: C0 = DT * w2 @ whh0[:, :256]
    C0 = (DT * (w2.astype(np.float64) @ whh0[:, 0:256].astype(np.float64))).astype(np.float32)

    h0T = np.repeat(h0.reshape(DRNN, 1), R, axis=1).astype(np.float32)

    bp = np.zeros((DRNN, 24), np.float32)
    bp[:, 0], bp[:, 1] = b0[0:128], b0[128:256]
    bp[:, 2], bp[:, 3] = b1[0:128], b1[128:256]
    bp[:, 4] = DT * b2
    brz0 = bih0 + bhh0
    bp[:, 5], bp[:, 6] = brz0[0:128], brz0[128:256]
    bp[:, 7] = bih0[256:384]
    bp[:, 8] = bhh0[256:384]
    brz1 = bih1 + bhh1
    bp[:, 9], bp[:, 10] = brz1[0:128], brz1[128:256]
    bp[:, 11] = bih1[256:384]
    bp[:, 12] = bhh1[256:384]
    bp[0:DOUT, 15] = bout
    zb = DT * (b2 @ w0)
    for k in range(K):
        bp[:, 16 + 2 * k + 0] = b0[0:128] + k * zb[0:128]
        bp[:, 16 + 2 * k + 1] = b0[128:256] + k * zb[128:256]

    flags = (
        bool(np.any(b0) or np.any(b1) or np.any(b2)),
        bool(np.any(b2)),
        bool(np.any(brz0[0:256])),
        bool(np.any(bhh0[256:384])),
        bool(np.any(brz1[0:256])),
        bool(np.any(bhh1[256:384])),
        bool(np.any(bout)),
    )

    C = np.ascontiguousarray
    shared = {
        "h0f": h0T,
        "w0": C(w0),
        "w1a": C(w1[0:128]),
        "w1b": C(w1[128:256]),
        "w2a": C(DT * w2[0:128]),
        "w2b": C(DT * w2[128:256]),
        "W20a": C(W20[0:128]),
        "W20b": C(W20[128:256]),
        "C0a": C(C0[0:128]),
        "C0b": C(C0[128:256]),
        "wih0": C(wih0),
        "whh0": C(whh0),
        "wih1": C(wih1),
        "whh1": C(whh1),
        "wout": C(wout),
        "biaspk": bp,
    }

    in_maps = []
    for c in range(NCORES):
        xc = xs[c * BL:(c + 1) * BL]           # (BL, NS, N, DIN)
        xmT = xc.transpose(3, 1, 0, 2).reshape(DIN, SR)
        mc = ms[c * BL:(c + 1) * BL]           # (BL, NS, N)
        mrow = mc.transpose(1, 0, 2).reshape(1, SR)
        mbc = np.broadcast_to(mrow, (DRNN, SR))
        m = dict(shared)
        m["xm"] = np.ascontiguousarray(xmT, np.float32)
        m["mbc"] = _to_bf(mbc)
        in_maps.append(m)
    return in_maps, flags


def kernel(**inputs):
    in_maps, flags = _prep(inputs)
    if flags not in _prog_cache:
        _prog_cache[flags] = _build_program(flags)
    nc = _prog_cache[flags]

    from concourse.bass_utils import run_bass_kernel_spmd
    res = run_bass_kernel_spmd(nc, in_maps, core_ids=list(range(NCORES)))
    global _last_results
    _last_results = res.results

    ys = np.zeros((B, NSTEPS, P, J, DOUT), np.float32)
    for c in range(NCORES):
        y = res.results[c]["y"]                      # (NSTEPS, DOUT, R)
        y = y.reshape(NSTEPS, DOUT, BL, N).transpose(2, 0, 3, 1)
        ys[c * BL:(c + 1) * BL] = y.reshape(BL, NSTEPS, P, J, DOUT)
    return ys
